# revision 5
# baseline (speedup 1.0000x reference)
"""Trainium2 Bass kernel for LoRA linear: y = x @ (W + 2*B@A).T + b.

Full inputs: x (8, 2048, 2048) f32, W (2048, 2048) f32, b (2048,) f32,
B (2048, 16) f32, A (16, 2048) f32.  Output (8, 2048, 2048) f32.

Sharding: data-parallel over the batch dim — core i computes
y[i] = x[i] @ w.T + b with the merged weight w = W + 2*B@A.

Per-core kernel (bf16 TensorEngine compute, f32 accumulate):
  All x/W transposes ride the DMA xbar (SBUF->SBUF, one descriptor
  per [128, 2048] tile, ALL on the sync HWDGE queue — concurrent
  transposes on different queues corrupt each other).  The xbar's
  16x128 tiling lands row d of the transposed matrix at
  [partition d//16, slot d%16]; the SAME permutation applies to xT
  and wT, so the GEMM contraction over d is unaffected.

  phase 0: A/B f32 on the scalar HWDGE queue + ScalarE cast (the
           software DGE has a ~12us cold start), bias broadcast-cast
           on gpsimd, 2*B.T via PE transposes.
  phase W: W row-blocks f32 on scalar (8) / sync (3), cast-bf16 on
           gpsimd (5, late blocks); rank-16 delta matmul in f32 PSUM;
           DVE merge w16 = bf16(wrow + delta); one xbar transpose
           into wT.  No DRAM round trip, no all-stores barrier.
  phase x: x0 f32 on scalar + ScalarE cast; x1.. cast-loaded on
           gpsimd; one xbar transpose each into xT.
  main:    per (row tile, 512-col bank): 16 bf16 matmuls into PSUM,
           VectorE adds the bias during eviction, per-bank store on
           the sync queue (keeps the tail short and ys tiles small).
"""

import numpy as np

import concourse.bacc as bacc
import concourse.mybir as mybir
import concourse.tile as tile
from concourse import masks
from concourse.bass_utils import run_bass_kernel_spmd

N_CORES = 8
BATCH, S, D = 8, 2048, 2048
RANK = 16
SCALE = 2.0  # alpha / rank = 32 / 16
P = 128  # partitions
FREE = 512  # f32 elems per PSUM bank
ND = D // P  # 16 contraction tiles
NS = S // P  # 16 row tiles per core
NO = D // FREE  # 4 output banks per row tile
NG = ND // 4  # 4 groups of 4

F32 = mybir.dt.float32
BF16 = mybir.dt.bfloat16

# W row-block -> DMA queue: scalar carries 8, sync 3 (it also runs all
# the xbar transposes), gpsimd cast-loads 5 late blocks
W_QUEUE = {
    0: "scalar", 2: "scalar", 4: "scalar", 6: "scalar",
    8: "scalar", 10: "scalar", 12: "scalar", 14: "scalar",
    1: "sync", 3: "sync", 5: "sync",
    7: "gpsimd", 9: "gpsimd", 11: "gpsimd", 13: "gpsimd", 15: "gpsimd",
}


def build_nc():
    nc = bacc.Bacc(
        "TRN2", target_bir_lowering=False, debug=False, num_devices=N_CORES
    )
    x_d = nc.dram_tensor("x", [S, D], F32, kind="ExternalInput").ap()
    W_d = nc.dram_tensor("W", [D, D], F32, kind="ExternalInput").ap()
    b_d = nc.dram_tensor("b", [D], F32, kind="ExternalInput").ap()
    B_d = nc.dram_tensor("B", [D, RANK], F32, kind="ExternalInput").ap()
    A_d = nc.dram_tensor("A", [RANK, D], F32, kind="ExternalInput").ap()
    out_d = nc.dram_tensor("out", [S, D], F32, kind="ExternalOutput").ap()

    with tile.TileContext(nc) as tc:
        with (
            tc.tile_pool(name="singles", bufs=1) as singles,
            tc.tile_pool(name="wt", bufs=1) as wtp,
        ):
            ident = singles.tile([P, P], BF16)
            masks.make_identity(nc, ident[:])

            # A/B f32 on the scalar HWDGE queue, ScalarE casts to bf16
            Af = singles.tile([RANK, D], F32)
            nc.scalar.dma_start(out=Af[:], in_=A_d[:])
            Bf = singles.tile([P, ND * RANK], F32)
            nc.scalar.dma_start(
                out=Bf[:], in_=B_d.rearrange("(t p) r -> p t r", p=P)
            )
            A_sb = singles.tile([RANK, D], BF16)
            nc.scalar.copy(A_sb[:], Af[:])
            Bs = singles.tile([P, ND * RANK], BF16)
            nc.scalar.copy(Bs[:], Bf[:])
            B2T = singles.tile([RANK, D], BF16)

            # bias replicated across partitions: broadcast cast-DMA on
            # gpsimd (first in that queue; needed by the first y evict)
            bb = singles.tile([P, D], BF16)
            nc.gpsimd.dma_start(out=bb[:], in_=b_d[None, :].broadcast_to([P, D]))

            # merged transposed weight, bf16, xbar row permutation:
            # wT[p, t, o] = w[o, p*16 + t]
            wT = wtp.tile([P, ND, D], BF16)

            with (
                tc.tile_pool(name="wrow", bufs=5) as wrowp,
                tc.tile_pool(name="w16", bufs=3) as w16p,
                tc.tile_pool(name="xstage", bufs=4) as xstage,
                tc.tile_pool(name="xTp", bufs=6) as xTp,
                tc.tile_pool(name="yout", bufs=6) as youtp,
                tc.tile_pool(name="dpsum", bufs=4, space="PSUM") as dpsum,
                tc.tile_pool(name="btsum", bufs=1, space="PSUM") as btsum,
                tc.tile_pool(name="gpsum", bufs=3, space="PSUM") as gpsum,
            ):
                # x0 staged f32 via scalar + ScalarE cast (beats the
                # software-DGE cold start); x1.. cast-load on gpsimd
                x0f = singles.tile([P, D], F32)
                nc.scalar.dma_start(out=x0f[:], in_=x_d[0:P, :])
                xs0 = xstage.tile([P, D], BF16, tag="xs")
                nc.scalar.copy(xs0[:], x0f[:])

                def x_load(st):
                    xs = xstage.tile([P, D], BF16, tag="xs")
                    nc.gpsimd.dma_start(
                        out=xs[:], in_=x_d[st * P : (st + 1) * P, :]
                    )
                    return xs

                xs12 = [x_load(1), x_load(2)]

                # W loads, three queues
                wrows = []
                for ot in range(ND):
                    q = W_QUEUE[ot]
                    if q == "gpsimd":
                        wrow = wrowp.tile([P, D], BF16, tag="wrow")
                        nc.gpsimd.dma_start(
                            out=wrow[:], in_=W_d[ot * P : (ot + 1) * P, :]
                        )
                    else:
                        wrow = wrowp.tile([P, D], F32, tag="wrow")
                        eng = nc.scalar if q == "scalar" else nc.sync
                        eng.dma_start(
                            out=wrow[:], in_=W_d[ot * P : (ot + 1) * P, :]
                        )
                    wrows.append(wrow)

                def x_transpose(xs):
                    # xT[p, t, s] = x[s, p*16 + t]
                    xT = xTp.tile([P, ND, P], BF16, tag="xT")
                    nc.sync.dma_start_transpose(out=xT[:], in_=xs[:])
                    return xT

                xTs = [x_transpose(xs0)]

                # 2*B.T from the staged B tiles (PE transposes, tiny)
                for g in range(NG):
                    bps = btsum.tile([RANK, 4 * P], BF16, tag="bt")
                    for j in range(4):
                        t = 4 * g + j
                        nc.tensor.matmul(
                            bps[:, j * P : (j + 1) * P],
                            Bs[:, t * RANK : (t + 1) * RANK],
                            ident[:],
                            is_transpose=True,
                            start=(j == 0),
                            stop=(j == 3),
                        )
                    nc.vector.tensor_scalar_mul(
                        B2T[:, g * 4 * P : (g + 1) * 4 * P], bps[:], SCALE
                    )

                xTs.append(x_transpose(xs12[0]))
                xTs.append(x_transpose(xs12[1]))

                # ---- merged-weight build ----
                def w_compute(ot):
                    w16 = w16p.tile([P, D], BF16, tag="w16")
                    dps = [
                        dpsum.tile([P, FREE], F32, tag="dp", name=f"dp{ot}_{g}")
                        for g in range(NG)
                    ]
                    for g in range(NG):
                        nc.tensor.matmul(
                            dps[g][:],
                            B2T[:, ot * P : (ot + 1) * P],
                            A_sb[:, g * FREE : (g + 1) * FREE],
                            start=True,
                            stop=True,
                        )
                    for g in range(NG):
                        nc.vector.tensor_add(
                            w16[:, g * FREE : (g + 1) * FREE],
                            dps[g][:],
                            wrows[ot][:, g * FREE : (g + 1) * FREE],
                        )
                    nc.sync.dma_start_transpose(
                        out=wT[:, :, ot * P : (ot + 1) * P], in_=w16[:]
                    )

                def x_chain(st):
                    return x_transpose(x_load(st))

                for ot in range(ND):
                    w_compute(ot)
                    if ot in (7, 11, 15):
                        xTs.append(x_chain(len(xTs)))
                PRE = len(xTs)  # 6

                # ---- main loop: y = x @ wT + b ----
                for st in range(NS):
                    if st + PRE < NS:
                        xTs.append(x_chain(st + PRE))
                    xT = xTs[st]
                    for oc in range(NO):
                        gp = gpsum.tile([P, FREE], F32)
                        for dt in range(ND):
                            nc.tensor.matmul(
                                gp[:],
                                xT[:, dt, :],
                                wT[:, dt, oc * FREE : (oc + 1) * FREE],
                                start=(dt == 0),
                                stop=(dt == ND - 1),
                            )
                        ys = youtp.tile([P, FREE], F32, tag="ys")
                        nc.vector.tensor_add(
                            ys[:], gp[:], bb[:, oc * FREE : (oc + 1) * FREE]
                        )
                        nc.sync.dma_start(
                            out=out_d[
                                st * P : (st + 1) * P,
                                oc * FREE : (oc + 1) * FREE,
                            ],
                            in_=ys[:],
                        )

    nc.compile()
    return nc


_NC_CACHE = None


def _get_nc():
    global _NC_CACHE
    if _NC_CACHE is None:
        _NC_CACHE = build_nc()
    return _NC_CACHE


def make_in_maps(x, W, b, B, A):
    x = np.ascontiguousarray(x, dtype=np.float32)
    W = np.ascontiguousarray(W, dtype=np.float32)
    b = np.ascontiguousarray(b, dtype=np.float32)
    B = np.ascontiguousarray(B, dtype=np.float32)
    A = np.ascontiguousarray(A, dtype=np.float32)
    return [
        {"x": x[i], "W": W, "b": b, "B": B, "A": A} for i in range(N_CORES)
    ]


def run(inputs, **spmd_kwargs):
    """Run the SPMD kernel; returns (output, BassKernelResults)."""
    nc = _get_nc()
    in_maps = make_in_maps(**inputs)
    res = run_bass_kernel_spmd(nc, in_maps, core_ids=list(range(N_CORES)), **spmd_kwargs)
    out = np.stack([res.results[i]["out"] for i in range(N_CORES)]).astype(np.float32)
    return out, res


def kernel(x, W, b, B, A):
    out, _ = run({"x": x, "W": W, "b": b, "B": B, "A": A})
    return out


# revision 6
# speedup vs baseline: 1.2520x; 1.2520x over previous
"""Trainium2 Bass kernel for LoRA linear: y = x @ (W + 2*B@A).T + b.

Full inputs: x (8, 2048, 2048) f32, W (2048, 2048) f32, b (2048,) f32,
B (2048, 16) f32, A (16, 2048) f32.  Output (8, 2048, 2048) f32.

Sharding: data-parallel over the batch dim — core i computes
y[i] = x[i] @ w.T + b with the merged weight w = W + 2*B@A.

Per-core kernel (bf16 TensorEngine compute, f32 accumulate):
  phase 0: A/B staged f32 on the scalar HWDGE queue + ScalarE cast
           (the software DGE has a ~12us cold start), bias broadcast
           cast-DMA first on gpsimd, 2*B.T via PE transposes.
  phase W: W row-blocks preloaded on three queues (scalar 8 f32,
           sync 3 f32, gpsimd 5 cast-bf16); rank-16 delta matmul in
           f32 PSUM; DVE merge w16 = bf16(wrow + delta); 16 PE
           transposes of the merged rows, DVE evicts into wT.
           No DRAM round trip and no all-stores barrier (the old
           xbar scheme serialized ~60us mid-kernel; SBUF->SBUF xbar
           transposes shatter into tiny packets and are far slower
           than PE transposes on real HW).
  phase x (interleaved): x0 f32 on scalar + ScalarE cast; x1.. cast
           loads on gpsimd; 16 PE transposes each, ScalarE evicts.
  main:    per (row tile, 512-col bank): 16 bf16 matmuls into PSUM,
           VectorE adds the bias during eviction, per-bank stores on
           the sync queue (short tail, small ys tiles).
"""

import numpy as np

import concourse.bacc as bacc
import concourse.mybir as mybir
import concourse.tile as tile
from concourse import masks
from concourse.bass_utils import run_bass_kernel_spmd

N_CORES = 8
BATCH, S, D = 8, 2048, 2048
RANK = 16
SCALE = 2.0  # alpha / rank = 32 / 16
P = 128  # partitions
FREE = 512  # f32 elems per PSUM bank
ND = D // P  # 16 contraction tiles
NS = S // P  # 16 row tiles per core
NO = D // FREE  # 4 output banks per row tile
NG = ND // 4  # 4 groups of 4

F32 = mybir.dt.float32
BF16 = mybir.dt.bfloat16

W_QUEUE = {
    0: "scalar", 2: "scalar", 4: "scalar", 6: "scalar",
    8: "scalar", 10: "scalar", 12: "scalar", 14: "scalar",
    1: "sync", 3: "sync", 5: "sync",
    7: "gpsimd", 9: "gpsimd", 11: "gpsimd", 13: "gpsimd", 15: "gpsimd",
}


def build_nc():
    nc = bacc.Bacc(
        "TRN2", target_bir_lowering=False, debug=False, num_devices=N_CORES
    )
    x_d = nc.dram_tensor("x", [S, D], F32, kind="ExternalInput").ap()
    W_d = nc.dram_tensor("W", [D, D], F32, kind="ExternalInput").ap()
    b_d = nc.dram_tensor("b", [D], F32, kind="ExternalInput").ap()
    B_d = nc.dram_tensor("B", [D, RANK], F32, kind="ExternalInput").ap()
    A_d = nc.dram_tensor("A", [RANK, D], F32, kind="ExternalInput").ap()
    out_d = nc.dram_tensor("out", [S, D], F32, kind="ExternalOutput").ap()

    with tile.TileContext(nc) as tc:
        with (
            tc.tile_pool(name="singles", bufs=1) as singles,
            tc.tile_pool(name="wt", bufs=1) as wtp,
        ):
            ident = singles.tile([P, P], BF16)
            masks.make_identity(nc, ident[:])

            # A/B f32 on the scalar HWDGE queue, ScalarE casts to bf16
            Af = singles.tile([RANK, D], F32)
            nc.scalar.dma_start(out=Af[:], in_=A_d[:])
            Bf = singles.tile([P, ND * RANK], F32)
            nc.scalar.dma_start(
                out=Bf[:], in_=B_d.rearrange("(t p) r -> p t r", p=P)
            )
            A_sb = singles.tile([RANK, D], BF16)
            nc.scalar.copy(A_sb[:], Af[:])
            Bs = singles.tile([P, ND * RANK], BF16)
            nc.scalar.copy(Bs[:], Bf[:])
            B2T = singles.tile([RANK, D], BF16)

            # bias replicated across partitions: broadcast cast-DMA,
            # first op on the gpsimd queue (needed by the first y evict)
            bb = singles.tile([P, D], BF16)
            nc.gpsimd.dma_start(out=bb[:], in_=b_d[None, :].broadcast_to([P, D]))

            # merged transposed weight, bf16: wT[p, dt, o] = w[o, dt*128+p]
            wT = wtp.tile([P, ND, D], BF16)

            with (
                tc.tile_pool(name="wrow", bufs=5) as wrowp,
                tc.tile_pool(name="w16", bufs=3) as w16p,
                tc.tile_pool(name="xstage", bufs=4) as xstage,
                tc.tile_pool(name="xTp", bufs=6) as xTp,
                tc.tile_pool(name="yout", bufs=6) as youtp,
                tc.tile_pool(name="dpsum", bufs=3, space="PSUM") as dpsum,
                tc.tile_pool(name="tpsum", bufs=3, space="PSUM") as tpsum,
                tc.tile_pool(name="gpsum", bufs=2, space="PSUM") as gpsum,
            ):
                # x0 staged f32 via scalar + ScalarE cast (beats the
                # software-DGE cold start); x1.. cast-load on gpsimd
                x0f = singles.tile([P, D], F32)
                nc.scalar.dma_start(out=x0f[:], in_=x_d[0:P, :])
                xs0 = xstage.tile([P, D], BF16, tag="xs")
                nc.scalar.copy(xs0[:], x0f[:])

                def x_load(st):
                    xs = xstage.tile([P, D], BF16, tag="xs")
                    nc.gpsimd.dma_start(
                        out=xs[:], in_=x_d[st * P : (st + 1) * P, :]
                    )
                    return xs

                xs12 = [x_load(1), x_load(2)]

                # W loads, three queues, all started up front
                wrows = []
                for ot in range(ND):
                    q = W_QUEUE[ot]
                    if q == "gpsimd":
                        wrow = wrowp.tile([P, D], BF16, tag="wrow")
                        nc.gpsimd.dma_start(
                            out=wrow[:], in_=W_d[ot * P : (ot + 1) * P, :]
                        )
                    else:
                        wrow = wrowp.tile([P, D], F32, tag="wrow")
                        eng = nc.scalar if q == "scalar" else nc.sync
                        eng.dma_start(
                            out=wrow[:], in_=W_d[ot * P : (ot + 1) * P, :]
                        )
                    wrows.append(wrow)

                def x_transpose(xs):
                    xT = xTp.tile([P, ND, P], BF16, tag="xT")
                    # 8 transposes per bf16 PSUM bank, one ScalarE evict each
                    for g in range(2):
                        tp = tpsum.tile([P, 8 * P], BF16, tag="tp")
                        for j in range(8):
                            dt = 8 * g + j
                            nc.tensor.matmul(
                                tp[:, j * P : (j + 1) * P],
                                xs[:, dt * P : (dt + 1) * P],
                                ident[:],
                                is_transpose=True,
                                start=(j == 0),
                                stop=(j == 7),
                            )
                        nc.scalar.copy(xT[:, 8 * g : 8 * (g + 1), :], tp[:])
                    return xT

                xTs = [x_transpose(xs0)]

                # 2*B.T from the staged B tiles (PE transposes, tiny)
                for g in range(NG):
                    bps = tpsum.tile([RANK, 4 * P], BF16, tag="tp")
                    for j in range(4):
                        t = 4 * g + j
                        nc.tensor.matmul(
                            bps[:, j * P : (j + 1) * P],
                            Bs[:, t * RANK : (t + 1) * RANK],
                            ident[:],
                            is_transpose=True,
                            start=(j == 0),
                            stop=(j == 3),
                        )
                    nc.vector.tensor_scalar_mul(
                        B2T[:, g * 4 * P : (g + 1) * 4 * P], bps[:], SCALE
                    )

                xTs.append(x_transpose(xs12[0]))
                xTs.append(x_transpose(xs12[1]))

                # ---- merged-weight build ----
                def w_compute(ot):
                    w16 = w16p.tile([P, D], BF16, tag="w16")
                    dps = [
                        dpsum.tile([P, FREE], F32, tag="dp", name=f"dp{ot}_{g}")
                        for g in range(NG)
                    ]
                    for g in range(NG):
                        nc.tensor.matmul(
                            dps[g][:],
                            B2T[:, ot * P : (ot + 1) * P],
                            A_sb[:, g * FREE : (g + 1) * FREE],
                            start=True,
                            stop=True,
                        )
                    for g in range(NG):
                        nc.vector.tensor_add(
                            w16[:, g * FREE : (g + 1) * FREE],
                            dps[g][:],
                            wrows[ot][:, g * FREE : (g + 1) * FREE],
                        )
                    # transpose merged rows: w16[o, dt*128+q] ->
                    # wT[q, dt, ot*128 + o]; DVE evicts (ScalarE is busy
                    # with the x evicts)
                    for g in range(2):
                        tp = tpsum.tile([P, 8 * P], BF16, tag="tp")
                        for j in range(8):
                            dt = 8 * g + j
                            nc.tensor.matmul(
                                tp[:, j * P : (j + 1) * P],
                                w16[:, dt * P : (dt + 1) * P],
                                ident[:],
                                is_transpose=True,
                                start=(j == 0),
                                stop=(j == 7),
                            )
                        nc.vector.tensor_scalar_mul(
                            wT[:, 8 * g : 8 * (g + 1), ot * P : (ot + 1) * P],
                            tp[:],
                            1.0,
                        )

                def x_chain(st):
                    return x_transpose(x_load(st))

                for ot in range(ND):
                    w_compute(ot)
                    if ot in (7, 11, 15):
                        xTs.append(x_chain(len(xTs)))
                PRE = len(xTs)  # 6

                # ---- main loop: y = x @ wT + b ----
                for st in range(NS):
                    if st + PRE < NS:
                        xTs.append(x_chain(st + PRE))
                    xT = xTs[st]
                    for oc in range(NO):
                        gp = gpsum.tile([P, FREE], F32)
                        for dt in range(ND):
                            nc.tensor.matmul(
                                gp[:],
                                xT[:, dt, :],
                                wT[:, dt, oc * FREE : (oc + 1) * FREE],
                                start=(dt == 0),
                                stop=(dt == ND - 1),
                            )
                        ys = youtp.tile([P, FREE], F32, tag="ys")
                        nc.vector.tensor_add(
                            ys[:], gp[:], bb[:, oc * FREE : (oc + 1) * FREE]
                        )
                        nc.sync.dma_start(
                            out=out_d[
                                st * P : (st + 1) * P,
                                oc * FREE : (oc + 1) * FREE,
                            ],
                            in_=ys[:],
                        )

    nc.compile()
    return nc


_NC_CACHE = None


def _get_nc():
    global _NC_CACHE
    if _NC_CACHE is None:
        _NC_CACHE = build_nc()
    return _NC_CACHE


def make_in_maps(x, W, b, B, A):
    x = np.ascontiguousarray(x, dtype=np.float32)
    W = np.ascontiguousarray(W, dtype=np.float32)
    b = np.ascontiguousarray(b, dtype=np.float32)
    B = np.ascontiguousarray(B, dtype=np.float32)
    A = np.ascontiguousarray(A, dtype=np.float32)
    return [
        {"x": x[i], "W": W, "b": b, "B": B, "A": A} for i in range(N_CORES)
    ]


def run(inputs, **spmd_kwargs):
    """Run the SPMD kernel; returns (output, BassKernelResults)."""
    nc = _get_nc()
    in_maps = make_in_maps(**inputs)
    res = run_bass_kernel_spmd(nc, in_maps, core_ids=list(range(N_CORES)), **spmd_kwargs)
    out = np.stack([res.results[i]["out"] for i in range(N_CORES)]).astype(np.float32)
    return out, res


def kernel(x, W, b, B, A):
    out, _ = run({"x": x, "W": W, "b": b, "B": B, "A": A})
    return out


# revision 14
# speedup vs baseline: 1.2751x; 1.0185x over previous
"""Trainium2 Bass kernel for LoRA linear: y = x @ (W + 2*B@A).T + b.

Full inputs: x (8, 2048, 2048) f32, W (2048, 2048) f32, b (2048,) f32,
B (2048, 16) f32, A (16, 2048) f32.  Output (8, 2048, 2048) f32.

Sharding: data-parallel over the batch dim — core i computes
y[i] = x[i] @ w.T + b with the merged weight w = W + 2*B@A.

Per-core kernel (bf16 TensorEngine compute, f32 accumulate):
  phase 0: A/B staged f32 on the scalar HWDGE queue + ScalarE cast
           (the software DGE has a ~12us cold start), bias broadcast
           cast-DMA first on gpsimd, 2*B.T via PE transposes.
  phase W: W row-blocks preloaded on three queues (scalar 8 f32,
           sync 3 f32, gpsimd 5 cast-bf16); rank-16 delta matmul in
           f32 PSUM; DVE merge w16 = bf16(wrow + delta); 16 PE
           transposes of the merged rows, DVE evicts into wT.
           No DRAM round trip and no all-stores barrier (the old
           xbar scheme serialized ~60us mid-kernel; SBUF->SBUF xbar
           transposes shatter into tiny packets and are far slower
           than PE transposes on real HW).
  phase x (interleaved): x0 f32 on scalar + ScalarE cast; x1.. cast
           loads on gpsimd; 16 PE transposes each, ScalarE evicts.
  main:    per (row tile, 512-col bank): 16 bf16 matmuls into PSUM,
           VectorE adds the bias during eviction, per-bank stores on
           the sync queue (short tail, small ys tiles).
"""

import numpy as np

import concourse.bacc as bacc
import concourse.mybir as mybir
import concourse.tile as tile
from concourse import masks
from concourse.bass_utils import run_bass_kernel_spmd

N_CORES = 8
BATCH, S, D = 8, 2048, 2048
RANK = 16
SCALE = 2.0  # alpha / rank = 32 / 16
P = 128  # partitions
FREE = 512  # f32 elems per PSUM bank
ND = D // P  # 16 contraction tiles
NS = S // P  # 16 row tiles per core
NO = D // FREE  # 4 output banks per row tile
NG = ND // 4  # 4 groups of 4

F32 = mybir.dt.float32
BF16 = mybir.dt.bfloat16

W_QUEUE = {
    0: "scalar", 2: "scalar", 4: "scalar", 6: "scalar",
    8: "scalar", 10: "scalar", 12: "scalar", 14: "scalar",
    1: "sync", 3: "sync", 5: "sync",
    7: "gpsimd", 9: "gpsimd", 11: "gpsimd", 13: "gpsimd", 15: "gpsimd",
}


def build_nc():
    nc = bacc.Bacc(
        "TRN2", target_bir_lowering=False, debug=False, num_devices=N_CORES
    )
    x_d = nc.dram_tensor("x", [S, D], F32, kind="ExternalInput").ap()
    W_d = nc.dram_tensor("W", [D, D], F32, kind="ExternalInput").ap()
    b_d = nc.dram_tensor("b", [D], F32, kind="ExternalInput").ap()
    B_d = nc.dram_tensor("B", [D, RANK], F32, kind="ExternalInput").ap()
    A_d = nc.dram_tensor("A", [RANK, D], F32, kind="ExternalInput").ap()
    out_d = nc.dram_tensor("out", [S, D], F32, kind="ExternalOutput").ap()

    with tile.TileContext(nc) as tc:
        with (
            tc.tile_pool(name="singles", bufs=1) as singles,
            tc.tile_pool(name="wt", bufs=1) as wtp,
        ):
            ident = singles.tile([P, P], BF16)
            masks.make_identity(nc, ident[:])

            # A/B f32 on the scalar HWDGE queue, ScalarE casts to bf16.
            # B is loaded FLAT ([128, 256] f32, 1KB/partition contiguous) —
            # a "(t p) r -> p t r" gather generates 2048x64B descriptors
            # and hogs the queue head for ~16us.  Flat means partition p
            # holds B rows [16p, 16p+16), so the PE transposes below give
            # B.T columns o = p*16 + t; the strided evict view puts them
            # back in natural order.
            Af = singles.tile([RANK, D], F32)
            nc.scalar.dma_start(out=Af[:], in_=A_d[:])
            Bf = singles.tile([P, ND * RANK], F32)
            nc.scalar.dma_start(
                out=Bf[:], in_=B_d.rearrange("(p i) r -> p (i r)", p=P)
            )
            A_sb = singles.tile([RANK, D], BF16)
            nc.scalar.copy(A_sb[:], Af[:])
            Bs = singles.tile([P, ND * RANK], BF16)
            nc.scalar.copy(Bs[:], Bf[:])
            B2T = singles.tile([RANK, D], BF16)

            # bias replicated across partitions (broadcast cast-DMA on
            # gpsimd, emitted after x0 below — needed by the first y evict)
            bb = singles.tile([P, D], BF16)

            # merged transposed weight, bf16: wT[p, dt, o] = w[o, dt*128+p]
            wT = wtp.tile([P, ND, D], BF16)

            with (
                tc.tile_pool(name="wrow", bufs=6) as wrowp,
                tc.tile_pool(name="w16", bufs=3) as w16p,
                tc.tile_pool(name="xstage", bufs=4) as xstage,
                tc.tile_pool(name="xTp", bufs=6) as xTp,
                tc.tile_pool(name="yout", bufs=8) as youtp,
                tc.tile_pool(name="dpsum", bufs=3, space="PSUM") as dpsum,
                tc.tile_pool(name="tpsum", bufs=3, space="PSUM") as tpsum,
                tc.tile_pool(name="gpsum", bufs=2, space="PSUM") as gpsum,
            ):
                def x_load(st):
                    xs = xstage.tile([P, D], BF16, tag="xs")
                    nc.gpsimd.dma_start(
                        out=xs[:], in_=x_d[st * P : (st + 1) * P, :]
                    )
                    return xs

                # x0 first in the gpsimd queue, then bias, then x1/x2
                xs0 = x_load(0)
                nc.gpsimd.dma_start(
                    out=bb[:], in_=b_d[None, :].broadcast_to([P, D])
                )
                xs12 = [x_load(1), x_load(2)]

                # W loads, three queues, all started up front
                wrows = []
                for ot in range(ND):
                    q = W_QUEUE[ot]
                    if q == "gpsimd":
                        wrow = wrowp.tile([P, D], BF16, tag="wrow")
                        nc.gpsimd.dma_start(
                            out=wrow[:], in_=W_d[ot * P : (ot + 1) * P, :]
                        )
                    else:
                        wrow = wrowp.tile([P, D], F32, tag="wrow")
                        eng = nc.scalar if q == "scalar" else nc.sync
                        eng.dma_start(
                            out=wrow[:], in_=W_d[ot * P : (ot + 1) * P, :]
                        )
                    wrows.append(wrow)

                def x_transpose(xs):
                    xT = xTp.tile([P, ND, P], BF16, tag="xT")
                    # 8 transposes per bf16 PSUM bank, one ScalarE evict each
                    for g in range(2):
                        tp = tpsum.tile([P, 8 * P], BF16, tag="tp")
                        for j in range(8):
                            dt = 8 * g + j
                            nc.tensor.matmul(
                                tp[:, j * P : (j + 1) * P],
                                xs[:, dt * P : (dt + 1) * P],
                                ident[:],
                                is_transpose=True,
                                start=(j == 0),
                                stop=(j == 7),
                            )
                        nc.scalar.copy(xT[:, 8 * g : 8 * (g + 1), :], tp[:])
                    return xT

                xTs = [x_transpose(xs0)]

                # 2*B.T from the flat-staged B: transpose t gives B.T
                # columns o = p*16 + t; the rearranged strided view of B2T
                # scatters them back so B2T columns are natural o order.
                B2Tv = B2T[:].rearrange("r (p t) -> r t p", t=ND)
                for g in range(2):
                    bps = tpsum.tile([RANK, 8 * P], BF16, tag="tp")
                    for j in range(8):
                        t = 8 * g + j
                        nc.tensor.matmul(
                            bps[:, j * P : (j + 1) * P],
                            Bs[:, t * RANK : (t + 1) * RANK],
                            ident[:],
                            is_transpose=True,
                            start=(j == 0),
                            stop=(j == 7),
                        )
                    nc.vector.tensor_scalar_mul(
                        B2Tv[:, 8 * g : 8 * (g + 1), :], bps[:], SCALE
                    )

                xTs.append(x_transpose(xs12[0]))
                xTs.append(x_transpose(xs12[1]))

                # ---- merged-weight build ----
                def w_compute(ot):
                    w16 = w16p.tile([P, D], BF16, tag="w16")
                    dps = [
                        dpsum.tile([P, FREE], F32, tag="dp", name=f"dp{ot}_{g}")
                        for g in range(NG)
                    ]
                    for g in range(NG):
                        nc.tensor.matmul(
                            dps[g][:],
                            B2T[:, ot * P : (ot + 1) * P],
                            A_sb[:, g * FREE : (g + 1) * FREE],
                            start=True,
                            stop=True,
                        )
                    for g in range(NG):
                        nc.vector.tensor_add(
                            w16[:, g * FREE : (g + 1) * FREE],
                            dps[g][:],
                            wrows[ot][:, g * FREE : (g + 1) * FREE],
                        )
                    # transpose merged rows: w16[o, dt*128+q] ->
                    # wT[q, dt, ot*128 + o]; DVE evicts (ScalarE is busy
                    # with the x evicts)
                    for g in range(2):
                        tp = tpsum.tile([P, 8 * P], BF16, tag="tp")
                        for j in range(8):
                            dt = 8 * g + j
                            nc.tensor.matmul(
                                tp[:, j * P : (j + 1) * P],
                                w16[:, dt * P : (dt + 1) * P],
                                ident[:],
                                is_transpose=True,
                                start=(j == 0),
                                stop=(j == 7),
                            )
                        nc.vector.tensor_scalar_mul(
                            wT[:, 8 * g : 8 * (g + 1), ot * P : (ot + 1) * P],
                            tp[:],
                            1.0,
                        )

                def x_chain(st):
                    return x_transpose(x_load(st))

                for ot in range(ND):
                    w_compute(ot)
                    if ot in (7, 11, 15):
                        xTs.append(x_chain(len(xTs)))
                PRE = len(xTs)  # 6

                # ---- main loop: y = x @ wT + b ----
                for st in range(NS):
                    if st + PRE < NS:
                        xTs.append(x_chain(st + PRE))
                    xT = xTs[st]
                    for oc in range(NO):
                        gp = gpsum.tile([P, FREE], F32)
                        for dt in range(ND):
                            nc.tensor.matmul(
                                gp[:],
                                xT[:, dt, :],
                                wT[:, dt, oc * FREE : (oc + 1) * FREE],
                                start=(dt == 0),
                                stop=(dt == ND - 1),
                            )
                        ys = youtp.tile([P, FREE], F32, tag="ys")
                        nc.vector.tensor_add(
                            ys[:], gp[:], bb[:, oc * FREE : (oc + 1) * FREE]
                        )
                        nc.sync.dma_start(
                            out=out_d[
                                st * P : (st + 1) * P,
                                oc * FREE : (oc + 1) * FREE,
                            ],
                            in_=ys[:],
                        )

    nc.compile()
    return nc


_NC_CACHE = None


def _get_nc():
    global _NC_CACHE
    if _NC_CACHE is None:
        _NC_CACHE = build_nc()
    return _NC_CACHE


def make_in_maps(x, W, b, B, A):
    x = np.ascontiguousarray(x, dtype=np.float32)
    W = np.ascontiguousarray(W, dtype=np.float32)
    b = np.ascontiguousarray(b, dtype=np.float32)
    B = np.ascontiguousarray(B, dtype=np.float32)
    A = np.ascontiguousarray(A, dtype=np.float32)
    return [
        {"x": x[i], "W": W, "b": b, "B": B, "A": A} for i in range(N_CORES)
    ]


def run(inputs, **spmd_kwargs):
    """Run the SPMD kernel; returns (output, BassKernelResults)."""
    nc = _get_nc()
    in_maps = make_in_maps(**inputs)
    res = run_bass_kernel_spmd(nc, in_maps, core_ids=list(range(N_CORES)), **spmd_kwargs)
    out = np.stack([res.results[i]["out"] for i in range(N_CORES)]).astype(np.float32)
    return out, res


def kernel(x, W, b, B, A):
    out, _ = run({"x": x, "W": W, "b": b, "B": B, "A": A})
    return out
